# revision 38
# baseline (speedup 1.0000x reference)
"""Trainium2 Bass kernel for a Tacotron-style decoder step (nn_Decoder).

Sharding: data-parallel over batch across 8 NeuronCores (8 batches/core),
all weights replicated. No collectives.

Math exploited (from the reference):
  - h0 = c0 = 0 for both MI-LSTM cells => the h@U matmuls and the
    alpha/beta2 terms vanish; gates = beta1*(x@W) + b.
  - zoneout (inference): h = 0.9*h_new; the 0.9 factors are folded into
    the next consumer's weight matrix on the host.
  - beta1 is folded into W on the host; stop_W is folded into frame_W.

Device-side layout: all small activations are kept transposed
([features, batch]) so per-feature biases are per-partition and the
batch dim (8) rides the free axis. The big attention matmuls run in
bf16 at the PE's full 1 column/cycle rate with fp32 PSUM accumulation;
softmax and all elementwise math are fp32.
"""

import os
import numpy as np
import ml_dtypes

import concourse.bass as bass
import concourse.tile as tile
from concourse import mybir
from concourse.bass import ts
from concourse.bass_utils import run_bass_kernel_spmd
from concourse.masks import make_identity

F32 = mybir.dt.float32
BF16 = mybir.dt.bfloat16
AF = mybir.ActivationFunctionType
BF = ml_dtypes.bfloat16

NCORES = 8
B = 64            # global batch
BL = B // NCORES  # batches per core = 8
T = 1024          # encoder timesteps
D = 512           # encoder dim
U = 1024          # lstm/attention units
PRE = 256         # prenet dim
FR = 160          # frame dim
FO = FR + 1       # frames + stop logit
NG = 2            # batch groups per core
GB = BL // NG     # batches per group = 4

_cached = None

# Walrus' CoreV3 codegen rejects instructions carrying more than 2 sync-wait
# commands ("Too many sync wait commands", e.g. on the Tile-exit Drain which
# waits on the global clock). Hoist excess waits onto standalone
# EventSemaphore ops inserted just before the instruction on the same engine
# -- semantically identical (engine blocks on the waits either way).
MAX_SYNC_WAITS = 1


def _split_sync_waits(nc, maxw=MAX_SYNC_WAITS):
    for bb_name, bass_bb in nc.bb_map.items():
        bb = bass_bb.bb
        insts = bb.instructions
        new = []
        changed = False
        for inst in insts:
            si = getattr(inst, "sync_info", None)
            if si is not None and si.on_wait and len(si.on_wait) > maxw:
                waits = list(si.on_wait)
                excess, keep = waits[:-maxw], waits[-maxw:]
                for i in range(0, len(excess), maxw):
                    nop = mybir.InstEventSemaphore(
                        name=nc.get_next_instruction_name(),
                        engine=inst.engine,
                        ins=[], outs=[],
                        sync_info=mybir.SyncInfo(
                            on_wait=excess[i : i + maxw], on_update=[]),
                    )
                    nc.register_instruction(nop, overwrite=True)
                    new.append(nop)
                inst.sync_info = mybir.SyncInfo(
                    on_wait=keep, on_update=si.on_update)
                changed = True
            new.append(inst)
        if changed:
            bb.instructions = new


def _build():
    nc = bass.Bass("TRN2")

    # ---- per-core DRAM I/O ----
    d_enc_t = nc.dram_tensor("enc_t", [BL, 4, 128, T], BF16, kind="ExternalInput")
    d_enc_n = nc.dram_tensor("enc_n", [BL, 8, 128, D], BF16, kind="ExternalInput")
    d_prevT = nc.dram_tensor("prevT", [2, 128, BL], F32, kind="ExternalInput")
    d_p1w = nc.dram_tensor("p1w", [2, 128, PRE], F32, kind="ExternalInput")
    d_p1b = nc.dram_tensor("p1b", [128, 2], F32, kind="ExternalInput")
    d_p2w = nc.dram_tensor("p2w", [2, 128, PRE], F32, kind="ExternalInput")
    d_p2b = nc.dram_tensor("p2b", [128, 2], F32, kind="ExternalInput")
    d_wq = nc.dram_tensor("wq", [2, 128, U], BF16, kind="ExternalInput")
    d_qb = nc.dram_tensor("qb", [128, 8], F32, kind="ExternalInput")
    d_wv = nc.dram_tensor("wv", [4, 128, U], BF16, kind="ExternalInput")
    d_av = nc.dram_tensor("av", [128, 8], BF16, kind="ExternalInput")
    d_l1w = nc.dram_tensor("l1w", [6, 128, 4 * U], BF16, kind="ExternalInput")
    d_l1b = nc.dram_tensor("l1b", [128, 32], F32, kind="ExternalInput")
    d_l2w = nc.dram_tensor("l2w", [8, 128, 4 * U], BF16, kind="ExternalInput")
    d_l2b = nc.dram_tensor("l2b", [128, 32], F32, kind="ExternalInput")
    d_fw = nc.dram_tensor("fw", [12, 128, FO], BF16, kind="ExternalInput")
    d_fb = nc.dram_tensor("fb", [128, 2], F32, kind="ExternalInput")
    d_frames = nc.dram_tensor("frames", [BL, FR], F32, kind="ExternalOutput")
    d_stop = nc.dram_tensor("stop", [BL, 1], F32, kind="ExternalOutput")

    debug = bool(int(os.environ.get("KERNEL_DEBUG", "0")))
    if debug:
        d_dbg_q = nc.dram_tensor("dbg_q", [128, 8, BL], F32, kind="ExternalOutput")
        d_dbg_attn = nc.dram_tensor("dbg_attn", [BL, T], F32, kind="ExternalOutput")
        d_dbg_ctx = nc.dram_tensor("dbg_ctx", [BL, D], F32, kind="ExternalOutput")
        d_dbg_li = nc.dram_tensor("dbg_li", [128, 6, BL], F32, kind="ExternalOutput")
        d_dbg_h1 = nc.dram_tensor("dbg_h1", [128, 8, BL], F32, kind="ExternalOutput")
        d_dbg_g1 = nc.dram_tensor("dbg_g1", [128, 32, BL], F32, kind="ExternalOutput")

    with tile.TileContext(nc) as tc:
        with (
            tc.tile_pool(name="consts", bufs=1) as consts,
            tc.tile_pool(name="enct", bufs=2) as enct_p,
            tc.tile_pool(name="encn", bufs=4) as encn_p,
            tc.tile_pool(name="wp", bufs=2) as wp,
            tc.tile_pool(name="l2c", bufs=8) as l2c_p,
            tc.tile_pool(name="small", bufs=1) as small,
            tc.tile_pool(name="vps", bufs=2, space="PSUM") as vpool,
            tc.tile_pool(name="scps", bufs=1, space="PSUM") as scpool,
            tc.tile_pool(name="spps", bufs=2, space="PSUM") as sppool,
        ):
            # ---------- constants / weights into SBUF ----------
            ident = consts.tile([128, 128], F32, tag="ident")
            make_identity(nc, ident)

            prevT = consts.tile([128, 2, BL], F32, tag="prevT")
            nc.sync.dma_start(prevT, d_prevT[:].rearrange("k p b -> p k b"))
            p1w = consts.tile([128, 2, PRE], F32, tag="p1w")
            nc.sync.dma_start(p1w, d_p1w[:].rearrange("k p m -> p k m"))
            p1b = consts.tile([128, 2], F32, tag="p1b")
            nc.sync.dma_start(p1b, d_p1b[:])
            p2w = consts.tile([128, 2, PRE], F32, tag="p2w")
            nc.sync.dma_start(p2w, d_p2w[:].rearrange("k p m -> p k m"))
            p2b = consts.tile([128, 2], F32, tag="p2b")
            nc.sync.dma_start(p2b, d_p2b[:])
            # startup queue order tuned so the first v-matmul's operands (wv,
            # enc_t[0]) arrive before the less urgent q-path weights
            wv = consts.tile([128, 4, U], BF16, tag="wv")
            nc.sync.dma_start(wv, d_wv[:].rearrange("k p m -> p k m"))
            enc_pref = {}
            et0 = enct_p.tile([128, 4, T], BF16, tag="enct")
            nc.sync.dma_start(et0, d_enc_t[0].rearrange("c p t -> p c t"))
            wq = consts.tile([128, 2, U], BF16, tag="wq")
            nc.sync.dma_start(wq, d_wq[:].rearrange("k p m -> p k m"))
            qb = consts.tile([128, 8], F32, tag="qb")
            nc.sync.dma_start(qb, d_qb[:])
            en0 = encn_p.tile([128, 8, D], BF16, tag="encn")
            nc.sync.dma_start(en0, d_enc_n[0].rearrange("c p d -> p c d"))
            enc_pref[0] = (et0, en0)
            av = consts.tile([128, 8], BF16, tag="av")
            nc.sync.dma_start(av, d_av[:])

            # ---------- prenet + query (transposed, f32 matmuls) ----------
            x1ps = sppool.tile([128, 2, BL], F32, tag="sp")
            for m in range(2):
                for k in range(2):
                    nc.tensor.matmul(
                        x1ps[:, m, :], p1w[:, k, ts(m, 128)], prevT[:, k, :],
                        start=(k == 0), stop=(k == 1),
                    )
            x1T = small.tile([128, 2, BL], F32, tag="x1T")
            for m in range(2):
                nc.scalar.activation(x1T[:, m, :], x1ps[:, m, :], AF.Relu,
                                     bias=p1b[:, m : m + 1])

            # lstm-input rhs tile: chunks 0-3 = contextT, 4-5 = prenet x2T
            li_rhs = consts.tile([128, 6, BL], BF16, tag="li_rhs")
            x2ps = sppool.tile([128, 2, BL], F32, tag="sp")
            for m in range(2):
                for k in range(2):
                    nc.tensor.matmul(
                        x2ps[:, m, :], p2w[:, k, ts(m, 128)], x1T[:, k, :],
                        start=(k == 0), stop=(k == 1),
                    )
            for m in range(2):
                nc.scalar.activation(li_rhs[:, 4 + m, :], x2ps[:, m, :], AF.Relu,
                                     bias=p2b[:, m : m + 1])

            qps = sppool.tile([128, 8, BL], F32, tag="sp")
            for m in range(8):
                for k in range(2):
                    nc.tensor.matmul(
                        qps[:, m, :], wq[:, k, ts(m, 128)], li_rhs[:, 4 + k, :],
                        start=(k == 0), stop=(k == 1),
                    )
            qT = consts.tile([128, 8, BL], F32, tag="qT")
            nc.vector.tensor_tensor(
                qT, qps, qb[:, :, None].to_broadcast((128, 8, BL)),
                mybir.AluOpType.add,
            )
            if debug:
                nc.gpsimd.dma_start(d_dbg_q[:], qT)

            # ---------- LSTM2 weights: full prefetch on the gpsimd queue ----
            # (keeps the sync queue free for the latency-critical enc loads)
            l2w_tiles = []
            for k in range(8):
                l2ck = l2c_p.tile([128, 4 * U], BF16, tag="l2c")
                nc.gpsimd.dma_start(l2ck, d_l2w[k])
                l2w_tiles.append(l2ck)
            l1w = consts.tile([128, 6, 4 * U], BF16, tag="l1w")

            # ---------- attention, per group of 4 batches ----------
            ctx_nat = consts.tile([BL, D], F32, tag="ctx_nat")
            for g in range(NG):
                sps = scpool.tile([128, 2, 512], F32, tag="scps")
                nc.vector.memset(sps, 0.0)
                enc_n_tiles = {}
                for j in range(GB):
                    b = g * GB + j
                    if b in enc_pref:
                        enc_t_sb, enc_n_sb = enc_pref[b]
                    else:
                        enc_t_sb = enct_p.tile([128, 4, T], BF16, tag="enct")
                        nc.sync.dma_start(
                            enc_t_sb, d_enc_t[b].rearrange("c p t -> p c t"))
                        enc_n_sb = encn_p.tile([128, 8, D], BF16, tag="encn")
                        nc.sync.dma_start(
                            enc_n_sb, d_enc_n[b].rearrange("c p d -> p c d"))
                    enc_n_tiles[j] = enc_n_sb

                    for m in range(8):
                        vps = vpool.tile([128, T], F32, tag="vps")
                        for k in range(4):
                            for n in range(2):
                                nc.tensor.matmul(
                                    vps[:, ts(n, 512)],
                                    wv[:, k, ts(m, 128)],
                                    enc_t_sb[:, k, ts(n, 512)],
                                    start=(k == 0), stop=(k == 3),
                                )
                        w_sb = wp.tile([128, T], BF16, tag="w")
                        nc.scalar.activation(w_sb, vps, AF.Tanh,
                                             bias=qT[:, m, b : b + 1])
                        for n in range(2):
                            nc.tensor.matmul(
                                sps[32 * j : 32 * j + 1, n, :],
                                av[:, m : m + 1],
                                w_sb[:, ts(n, 512)],
                                start=(m == 0), stop=(m == 7),
                                tile_position=(0, 32 * j),
                            )

                # softmax over T for the 4 batches (rows 0,32,64,96)
                negmax = small.tile([97, 1], F32, tag="negmax")
                nc.vector.tensor_reduce(
                    negmax, sps[:97], mybir.AxisListType.XY,
                    mybir.AluOpType.max, negate=True,
                )
                attn_sb = small.tile([97, 2, 512], F32, tag="attn")
                nc.scalar.activation(attn_sb, sps[:97], AF.Exp, bias=negmax)
                asum = small.tile([97, 1], F32, tag="asum")
                nc.vector.tensor_reduce(
                    asum, attn_sb, mybir.AxisListType.XY, mybir.AluOpType.add)
                rsum = small.tile([97, 1], F32, tag="rsum")
                nc.vector.reciprocal(rsum, asum)
                nc.vector.tensor_scalar_mul(attn_sb, attn_sb, rsum)

                # gather the 4 attn rows compactly, then transpose to [T, 4]
                attn_g = small.tile([GB, T], F32, tag="attn_g")
                nc.gpsimd.dma_start(attn_g, attn_sb[0:97:32])
                if debug:
                    nc.gpsimd.dma_start(d_dbg_attn[ts(g, GB)], attn_g)
                atps = sppool.tile([128, 8, GB], F32, tag="sp")
                for t in range(8):
                    nc.tensor.transpose(
                        atps[:, t, :], attn_g[:, ts(t, 128)], ident[:GB, :GB])
                attnT = small.tile([128, 8, GB], BF16, tag="attnT")
                nc.vector.tensor_copy(attnT, atps)

                # context for the group's 4 batches
                cps = sppool.tile([128, D], F32, tag="sp")
                nc.vector.memset(cps, 0.0)
                for j in range(GB):
                    for t in range(8):
                        nc.tensor.matmul(
                            cps[32 * j : 32 * j + 1, :],
                            attnT[:, t, j : j + 1],
                            enc_n_tiles[j][:, t, :],
                            start=(t == 0), stop=(t == 7),
                            tile_position=(0, 32 * j),
                        )
                ctx_g = small.tile([97, D], F32, tag="ctx_g")
                nc.vector.tensor_copy(ctx_g, cps[:97])
                nc.gpsimd.dma_start(ctx_nat[ts(g, GB), :], ctx_g[0:97:32])

                if g == 0:
                    # late weights: emitted here so the sync queue serves the
                    # first enc tiles before these bulk loads
                    for h in range(3):
                        nc.sync.dma_start(
                            l1w[:, ts(h, 2), :],
                            d_l1w[ts(h, 2)].rearrange("k p m -> p k m"),
                        )
                    fw = consts.tile([128, 12, FO], BF16, tag="fw")
                    nc.sync.dma_start(fw, d_fw[:].rearrange("k p m -> p k m"))
                    fb = consts.tile([128, 2], F32, tag="fb")
                    nc.sync.dma_start(fb, d_fb[:])
                    l1b = consts.tile([128, 32], F32, tag="l1b")
                    nc.sync.dma_start(l1b, d_l1b[:])
                    l2b = consts.tile([128, 32], F32, tag="l2b")
                    nc.sync.dma_start(l2b, d_l2b[:])

            # contextT -> li_rhs chunks 0..3
            ctps = sppool.tile([128, 4, BL], F32, tag="sp")
            for c in range(4):
                nc.tensor.transpose(
                    ctps[:, c, :], ctx_nat[:, ts(c, 128)], ident[:BL, :BL])
            nc.vector.tensor_copy(li_rhs[:, 0:4, :], ctps)
            if debug:
                nc.gpsimd.dma_start(d_dbg_ctx[:], ctx_nat)
                li_f32 = small.tile([128, 6, BL], F32, tag="li_f32")
                nc.vector.tensor_copy(li_f32, li_rhs)
                nc.gpsimd.dma_start(d_dbg_li[:], li_f32)

            # ---------- MI-LSTM cells (transposed: [feat, batch]) ----------
            def lstm(lhsT_slices, rhs_slices, bias, h_out, dbg_g=None):
                # Accumulation groups in one PSUM bank must be sequential
                # (start=True clears has_written bank-wide), so each m-group
                # runs its k-chain contiguously. Split k into two passes so
                # streamed weight chunks 4..7 can reuse the slots chunks 0..3
                # held during pass one.
                nk = len(lhsT_slices)
                half = (nk + 1) // 2
                parts = []
                for lo, hi in ((0, half), (half, nk)):
                    gp = sppool.tile([128, 32, BL], F32, tag="sp")
                    for m in range(32):
                        for k in range(lo, hi):
                            nc.tensor.matmul(
                                gp[:, m, :],
                                lhsT_slices[k][:, ts(m, 128)],
                                rhs_slices[k],
                                start=(k == lo), stop=(k == hi - 1),
                            )
                    parts.append(gp)
                # DVE may read only one PSUM operand per instruction
                g_sb = small.tile([128, 32, BL], F32, tag="gsb")
                nc.vector.tensor_tensor(
                    g_sb, parts[0], bias[:, :, None].to_broadcast((128, 32, BL)),
                    mybir.AluOpType.add,
                )
                nc.vector.tensor_tensor(
                    g_sb, g_sb, parts[1], mybir.AluOpType.add)
                if dbg_g is not None:
                    nc.gpsimd.dma_start(dbg_g[:], g_sb)
                nc.scalar.activation(g_sb[:, 0:16], g_sb[:, 0:16], AF.Sigmoid)
                nc.scalar.activation(g_sb[:, 24:32], g_sb[:, 24:32], AF.Sigmoid)
                nc.scalar.activation(g_sb[:, 16:24], g_sb[:, 16:24], AF.Tanh)
                ct = small.tile([128, 8, BL], F32, tag="ct")
                nc.vector.tensor_mul(ct, g_sb[:, 0:8], g_sb[:, 16:24])
                nc.scalar.activation(ct, ct, AF.Tanh)
                nc.vector.tensor_mul(h_out, g_sb[:, 24:32], ct)

            h1T = consts.tile([128, 8, BL], BF16, tag="h1T")
            lstm([l1w[:, k, :] for k in range(6)],
                 [li_rhs[:, k, :] for k in range(6)], l1b, h1T,
                 dbg_g=d_dbg_g1 if debug else None)
            if debug:
                h1_f32 = small.tile([128, 8, BL], F32, tag="h1_f32")
                nc.vector.tensor_copy(h1_f32, h1T)
                nc.gpsimd.dma_start(d_dbg_h1[:], h1_f32)
            h2T = consts.tile([128, 8, BL], BF16, tag="h2T")
            lstm(l2w_tiles, [h1T[:, k, :] for k in range(8)], l2b, h2T)

            # ---------- frame + stop projection ----------
            fps = sppool.tile([128, 2, BL], F32, tag="sp")
            for ms, mo, msz in ((0, 0, 128), (1, 128, FO - 128)):
                for k in range(12):
                    rhs = li_rhs[:, k, :] if k < 4 else h2T[:, k - 4, :]
                    nc.tensor.matmul(
                        fps[:msz, ms, :], fw[:, k, mo : mo + msz], rhs,
                        start=(k == 0), stop=(k == 11),
                    )
            fT = small.tile([128, 2, BL], F32, tag="fT")
            nc.scalar.activation(fT[:, 0, :], fps[:, 0, :], AF.Identity,
                                 bias=fb[:, 0:1])
            nc.scalar.activation(fT[:33, 1, :], fps[:33, 1, :], AF.Identity,
                                 bias=fb[:33, 1:2])
            tps = sppool.tile([BL, FO], F32, tag="sp")
            nc.tensor.transpose(tps[:, 0:128], fT[:, 0, :], ident)
            nc.tensor.transpose(tps[:, 128:FO], fT[:33, 1, :], ident[:33, :33])
            out_sb = small.tile([BL, FO], F32, tag="out")
            nc.vector.tensor_copy(out_sb, tps)
            nc.scalar.activation(out_sb[:, FR:FO], out_sb[:, FR:FO], AF.Sigmoid)
            nc.gpsimd.dma_start(d_frames[:], out_sb[:, 0:FR])
            nc.gpsimd.dma_start(d_stop[:], out_sb[:, FR:FO])

    _split_sync_waits(nc)
    nc.finalize()
    return nc


def _prep_inputs(inp):
    """Host-side layout prep. Returns per-core in_maps."""
    f32 = np.float32
    g = lambda k: np.asarray(inp[k], dtype=f32)

    enc = g("enc_out")  # [B, T, D]
    enc_t = np.ascontiguousarray(enc.transpose(0, 2, 1)).astype(BF).reshape(B, 4, 128, T)
    enc_n = enc.astype(BF).reshape(B, 8, 128, D)

    prev = g("prev_out")  # [B, FR]
    prevT = np.zeros((B // BL, 2 * 128, BL), f32)
    for c in range(NCORES):
        prevT[c, :FR, :] = prev[c * BL : (c + 1) * BL].T
    prevT = prevT.reshape(NCORES, 2, 128, BL)

    p1w = np.zeros((2 * 128, PRE), f32)
    p1w[:FR] = g("p1_W")
    p1w = p1w.reshape(2, 128, PRE)
    p1b = g("p1_b").reshape(2, 128).T.copy()
    p2w = g("p2_W").reshape(2, 128, PRE)
    p2b = g("p2_b").reshape(2, 128).T.copy()

    wq = g("att_Wq").astype(BF).reshape(2, 128, U)
    qb = (g("att_bq") + g("att_bv")).reshape(8, 128).T.copy()
    wv = g("att_Wv").astype(BF).reshape(4, 128, U)
    av = g("att_v")[:, 0].astype(BF).reshape(8, 128).T.copy()

    l1w = (g("l1_W") * g("l1_beta1")[None, :]).astype(BF).reshape(6, 128, 4 * U)
    l1b = g("l1_b").reshape(32, 128).T.copy()
    l2w = (0.9 * g("l2_W") * g("l2_beta1")[None, :]).astype(BF).reshape(8, 128, 4 * U)
    l2b = g("l2_b").reshape(32, 128).T.copy()

    fw = np.concatenate([g("frame_W"), g("stop_W")], axis=1)  # [1536, 161]
    fw[D:] *= 0.9  # zoneout on h2
    fw = fw.astype(BF).reshape(12, 128, FO)
    fbv = np.concatenate([g("frame_b"), g("stop_b")])  # [161]
    fb = np.zeros((128, 2), f32)
    fb[:, 0] = fbv[:128]
    fb[: FO - 128, 1] = fbv[128:]

    shared = dict(p1w=p1w, p1b=p1b, p2w=p2w, p2b=p2b, wq=wq, qb=qb, wv=wv,
                  av=av, l1w=l1w, l1b=l1b, l2w=l2w, l2b=l2b, fw=fw, fb=fb)
    in_maps = []
    for c in range(NCORES):
        m = dict(shared)
        m["enc_t"] = enc_t[c * BL : (c + 1) * BL]
        m["enc_n"] = enc_n[c * BL : (c + 1) * BL]
        m["prevT"] = prevT[c]
        in_maps.append(m)
    return in_maps


last_result = None


def kernel(**inputs):
    global _cached, last_result
    if _cached is None:
        _cached = _build()
    nc = _cached
    in_maps = _prep_inputs(inputs)
    trace = bool(int(os.environ.get("KERNEL_TRACE", "0")))
    res = run_bass_kernel_spmd(nc, in_maps, core_ids=list(range(NCORES)),
                               trace=trace)
    last_result = res
    frames = np.concatenate([r["frames"] for r in res.results], axis=0)
    stop = np.concatenate([r["stop"] for r in res.results], axis=0)
    return frames.astype(np.float32), stop.astype(np.float32)
